# revision 10
# baseline (speedup 1.0000x reference)
"""Trainium2 Bass kernel for the ragged-BOW model (EmbeddingBag-mean -> FC -> BN -> ReLU -> FC -> BCE).

Self-contained: hardcodes shapes B=4096, L=200, V=100000, D=256, 8 cores.
Data-parallel over batch; embedding table replicated (bf16), gathered via
dma_gather with int16 indices over 4 vocab chunks; segment mean-pool via
two-level mask matmuls on the tensor engine; BN batch stats all-reduced
across the 8 cores on device; loss partials summed on host.
"""
import sys

sys.path.insert(0, "/opt/trn_rl_repo")

import numpy as np
import ml_dtypes

from concourse import bass, bacc, mybir, tile
from concourse.bass_utils import run_bass_kernel_spmd

P = 128
B = 4096
L = 200
V = 100000
D = 256
NCORES = 8
BPC = B // NCORES          # 512 batch elements per core
NCHUNK = 4                 # vocab chunks (int16-indexable)
CHUNK = V // NCHUNK        # 25000
CHUNK_P = CHUNK + 1        # +1 zero row per chunk
ZROW = CHUNK               # local index of the zero row
BS = 8                     # rows per blocksum block
BLOCKS_PER_TILE = P // BS  # 16
TILES_PER_ROUND = P // BLOCKS_PER_TILE  # 8
ROWS_PER_ROUND = P * TILES_PER_ROUND    # 1024
MAX_CALL = 8192            # indices per dma_gather call
BN_EPS = 1e-5

f32 = mybir.dt.float32
bf16 = mybir.dt.bfloat16
i16 = mybir.dt.int16
i32 = mybir.dt.int32

_PROGRAM_CACHE = {}


def _build_program(R):
    """R: rows per (quarter) region, multiple of 1024. Stream = NCHUNK regions."""
    key = R
    if key in _PROGRAM_CACHE:
        return _PROGRAM_CACHE[key]

    NR = NCHUNK * R // ROWS_PER_ROUND      # total rounds (= level-2 tiles)
    rounds_per_region = R // ROWS_PER_ROUND

    nc = bacc.Bacc(None, num_devices=NCORES)

    table = nc.dram_tensor("table", [NCHUNK * CHUNK_P, D], bf16, kind="ExternalInput")
    idx = nc.dram_tensor("idx", [P, NCHUNK * R // 16], i16, kind="ExternalInput")
    seg = nc.dram_tensor("seg", [P, NR], f32, kind="ExternalInput")
    wts = nc.dram_tensor("wts", [P, NR], f32, kind="ExternalInput")
    w1t = nc.dram_tensor("w1t", [2 * P, D], f32, kind="ExternalInput")   # W1.T
    w2v = nc.dram_tensor("w2v", [P, 2], f32, kind="ExternalInput")
    gam = nc.dram_tensor("gam", [P, 2], f32, kind="ExternalInput")
    bet = nc.dram_tensor("bet", [P, 2], f32, kind="ExternalInput")
    tv = nc.dram_tensor("tv", [P, BPC // P], f32, kind="ExternalInput")
    b2v = nc.dram_tensor("b2v", [P, 1], f32, kind="ExternalInput")
    iotab = nc.dram_tensor("iotab", [P, BPC], f32, kind="ExternalInput")
    l1m = nc.dram_tensor("l1m", [P, TILES_PER_ROUND * P], bf16, kind="ExternalInput")

    logits_out = nc.dram_tensor("logits_s", [P, BPC // P], f32, kind="ExternalOutput")
    loss_out = nc.dram_tensor("loss_s", [1, 1], f32, kind="ExternalOutput")

    with tile.TileContext(nc) as tc:
        with (
            tc.tile_pool(name="const", bufs=1) as const,
            tc.tile_pool(name="gpool", bufs=3) as gpool,
            tc.tile_pool(name="spool", bufs=3) as spool,
            tc.tile_pool(name="psum", bufs=2, space="PSUM") as psum,
            tc.tile_pool(name="psum1", bufs=1, space="PSUM") as psum1,
            tc.tile_pool(name="dram", bufs=1, space="DRAM") as dram,
        ):
            # ---- constant loads -------------------------------------------
            idx_sb = const.tile([P, NCHUNK * R // 16], i16)
            nc.sync.dma_start(out=idx_sb[:], in_=idx[:])
            seg_sb = const.tile([P, NR], f32)
            nc.sync.dma_start(out=seg_sb[:], in_=seg[:])
            wts_sb = const.tile([P, NR], f32)
            nc.sync.dma_start(out=wts_sb[:], in_=wts[:])
            w1t_sb0 = const.tile([P, D], f32)
            nc.sync.dma_start(out=w1t_sb0[:], in_=w1t[0:P, :])
            w1t_sb1 = const.tile([P, D], f32)
            nc.sync.dma_start(out=w1t_sb1[:], in_=w1t[P:2 * P, :])
            w2_sb = const.tile([P, 2], f32)
            nc.sync.dma_start(out=w2_sb[:], in_=w2v[:])
            gam_sb = const.tile([P, 2], f32)
            nc.sync.dma_start(out=gam_sb[:], in_=gam[:])
            bet_sb = const.tile([P, 2], f32)
            nc.sync.dma_start(out=bet_sb[:], in_=bet[:])
            t_sb = const.tile([P, BPC // P], f32)
            nc.sync.dma_start(out=t_sb[:], in_=tv[:])
            b2_sb = const.tile([P, 1], f32)
            nc.sync.dma_start(out=b2_sb[:], in_=b2v[:])

            # ---- constant masks & iota row (uploaded from host) -----------
            iota_bf = const.tile([P, BPC], f32)
            nc.sync.dma_start(out=iota_bf[:], in_=iotab[:])
            l1m_sb = const.tile([P, TILES_PER_ROUND * P], bf16)
            nc.sync.dma_start(out=l1m_sb[:], in_=l1m[:])
            l1_masks = [l1m_sb[:, j * P:(j + 1) * P] for j in range(TILES_PER_ROUND)]

            ones_col = const.tile([P, 1], f32)
            nc.vector.memset(ones_col[:], 1.0)

            # ---- level-2 output accumulators ------------------------------
            pooled_ps0 = psum1.tile([P, BPC], f32, space="PSUM")
            pooled_ps1 = psum1.tile([P, BPC], f32, space="PSUM")

            # ---- main gather / reduce loop --------------------------------
            u = 0  # global round index
            for region in range(NCHUNK):
                base_row = region * R
                chunk_lo = region * CHUNK_P
                done = 0
                while done < R:
                    csz = min(MAX_CALL, R - done)
                    tiles_per_call = csz // P
                    gat = gpool.tile([P, MAX_CALL // P, D], bf16, tag="gat")
                    icol0 = (base_row + done) // 16
                    nc.gpsimd.dma_gather(
                        out_ap=gat[:, :tiles_per_call, :],
                        in_ap=table[chunk_lo:chunk_lo + CHUNK_P, :],
                        idxs_ap=idx_sb[:, icol0:icol0 + csz // 16],
                        num_idxs=csz,
                        num_idxs_reg=csz,
                        elem_size=D,
                        single_packet=False,
                    )
                    for rt in range(tiles_per_call // TILES_PER_ROUND):
                        acc = psum.tile([P, D], f32, space="PSUM", tag="acc")
                        for j in range(TILES_PER_ROUND):
                            c = rt * TILES_PER_ROUND + j
                            nc.tensor.matmul(
                                out=acc[:],
                                lhsT=l1_masks[j],
                                rhs=gat[:, c, :],
                                start=(j == 0),
                                stop=(j == TILES_PER_ROUND - 1),
                            )
                        # scaled copy PSUM->SBUF (x 1/len), cast to bf16
                        bsum = spool.tile([P, D], bf16, tag="bsum")
                        nc.vector.tensor_scalar(
                            out=bsum[:], in0=acc[:],
                            scalar1=wts_sb[:, u:u + 1], scalar2=None,
                            op0=mybir.AluOpType.mult,
                        )
                        # level-2 mask for this round
                        msk = spool.tile([P, BPC], bf16, tag="msk")
                        nc.vector.tensor_tensor(
                            out=msk[:],
                            in0=seg_sb[:, u:u + 1].to_broadcast([P, BPC]),
                            in1=iota_bf[:],
                            op=mybir.AluOpType.is_equal,
                        )
                        nc.tensor.matmul(
                            out=pooled_ps0[:],
                            lhsT=bsum[:, 0:P],
                            rhs=msk[:],
                            start=(u == 0),
                            stop=(u == NR - 1),
                        )
                        nc.tensor.matmul(
                            out=pooled_ps1[:],
                            lhsT=bsum[:, P:D],
                            rhs=msk[:],
                            start=(u == 0),
                            stop=(u == NR - 1),
                        )
                        u += 1
                    done += csz
            assert u == NR

            # ---- pooledT -> SBUF ------------------------------------------
            pooled_sb0 = const.tile([P, BPC], f32)
            nc.vector.tensor_copy(pooled_sb0[:], pooled_ps0[:])
            pooled_sb1 = const.tile([P, BPC], f32)
            nc.vector.tensor_copy(pooled_sb1[:], pooled_ps1[:])

            # ---- z.T = W1 @ pooled.T  (f32, b1 dropped: BN shift-invariant)
            z_ps = []
            for h in range(2):
                zp = psum1.tile([P, BPC], f32, space="PSUM", name=f"z_ps{h}")
                nc.tensor.matmul(
                    out=zp[:], lhsT=w1t_sb0[:, h * P:(h + 1) * P], rhs=pooled_sb0[:],
                    start=True, stop=False,
                )
                nc.tensor.matmul(
                    out=zp[:], lhsT=w1t_sb1[:, h * P:(h + 1) * P], rhs=pooled_sb1[:],
                    start=False, stop=True,
                )
                z_ps.append(zp)

            # ---- per-core BN stats: s1 = sum_b z, s2 = sum_b z^2 ----------
            z_sb = []
            stats_sb = const.tile([P, 4], f32)
            sq_scratch = spool.tile([P, BPC], f32, tag="sqs")
            for h in range(2):
                zs = const.tile([P, BPC], f32, name=f"z_sb{h}")
                nc.scalar.activation(
                    out=zs[:], in_=z_ps[h][:],
                    func=mybir.ActivationFunctionType.Identity,
                    accum_out=stats_sb[:, h:h + 1],
                )
                nc.scalar.activation(
                    out=sq_scratch[:], in_=z_ps[h][:],
                    func=mybir.ActivationFunctionType.Square,
                    accum_out=stats_sb[:, 2 + h:3 + h],
                )
                z_sb.append(zs)

            # ---- all-reduce stats across the 8 cores ----------------------
            ar_in = dram.tile([P, 4], f32)
            ar_out = dram.tile([P, 4], f32, addr_space="Shared")
            nc.sync.dma_start(out=ar_in[:], in_=stats_sb[:])
            nc.gpsimd.collective_compute(
                "AllReduce",
                mybir.AluOpType.add,
                replica_groups=[list(range(NCORES))],
                ins=[ar_in[:].opt()],
                outs=[ar_out[:].opt()],
            )
            stats_g = const.tile([P, 4], f32)
            nc.sync.dma_start(out=stats_g[:], in_=ar_out[:])

            # ---- BN scalars: a = gamma/std, cbias = beta - mu*a ----------
            mu = const.tile([P, 2], f32)
            nc.vector.tensor_scalar_mul(mu[:], stats_g[:, 0:2], 1.0 / B)
            ex2 = const.tile([P, 2], f32)
            nc.vector.tensor_scalar_mul(ex2[:], stats_g[:, 2:4], 1.0 / B)
            mu2 = const.tile([P, 2], f32)
            nc.vector.tensor_tensor(out=mu2[:], in0=mu[:], in1=mu[:], op=mybir.AluOpType.mult)
            var = const.tile([P, 2], f32)
            nc.vector.tensor_tensor(out=var[:], in0=ex2[:], in1=mu2[:], op=mybir.AluOpType.subtract)
            var_eps = const.tile([P, 2], f32)
            nc.vector.tensor_scalar_add(var_eps[:], var[:], BN_EPS)
            std = const.tile([P, 2], f32)
            nc.scalar.activation(
                out=std[:], in_=var_eps[:], func=mybir.ActivationFunctionType.Sqrt,
            )
            inv = const.tile([P, 2], f32)
            nc.vector.reciprocal(inv[:], std[:])
            a_sc = const.tile([P, 2], f32)
            nc.vector.tensor_tensor(out=a_sc[:], in0=gam_sb[:], in1=inv[:], op=mybir.AluOpType.mult)
            mua = const.tile([P, 2], f32)
            nc.vector.tensor_tensor(out=mua[:], in0=mu[:], in1=a_sc[:], op=mybir.AluOpType.mult)
            cb = const.tile([P, 2], f32)
            nc.vector.tensor_tensor(out=cb[:], in0=bet_sb[:], in1=mua[:], op=mybir.AluOpType.subtract)

            # ---- h.T = relu(a*z + cbias) ----------------------------------
            h_sb = []
            for h in range(2):
                hs = const.tile([P, BPC], f32, name=f"h_sb{h}")
                nc.scalar.activation(
                    out=hs[:], in_=z_sb[h][:],
                    func=mybir.ActivationFunctionType.Relu,
                    bias=cb[:, h:h + 1], scale=a_sc[:, h:h + 1],
                )
                h_sb.append(hs)

            # ---- logits[b] = h.T @ w2 + b2 --------------------------------
            logit_ps = psum1.tile([P, BPC // P], f32, space="PSUM")
            for j in range(BPC // P):
                for h in range(2):
                    nc.tensor.matmul(
                        out=logit_ps[:, j:j + 1],
                        lhsT=h_sb[h][:, j * P:(j + 1) * P],
                        rhs=w2_sb[:, h:h + 1],
                        start=(h == 0),
                        stop=(h == 1),
                    )
            l_sb = const.tile([P, BPC // P], f32)
            nc.scalar.activation(
                out=l_sb[:], in_=logit_ps[:],
                func=mybir.ActivationFunctionType.Identity,
                bias=b2_sb[:, :1], scale=1.0,
            )
            nc.sync.dma_start(out=logits_out[:], in_=l_sb[:])

            # ---- BCE: relu(l) - l*t + log1p(exp(-|l|)) --------------------
            # log1p has no ACT table; use ln(z) ~= 64*w - 32*w^2, w = z^(1/64)-1
            nchunk_b = BPC // P
            neg_l = const.tile([P, nchunk_b], f32)
            nc.vector.tensor_scalar_mul(neg_l[:], l_sb[:], -1.0)
            abs_l = const.tile([P, nchunk_b], f32)
            nc.vector.tensor_tensor(
                out=abs_l[:], in0=l_sb[:], in1=neg_l[:], op=mybir.AluOpType.max,
            )
            uexp = const.tile([P, nchunk_b], f32)
            nc.scalar.activation(
                out=uexp[:], in_=abs_l[:], func=mybir.ActivationFunctionType.Exp,
                scale=-1.0,
            )
            zp = const.tile([P, nchunk_b], f32)
            nc.vector.tensor_scalar_add(zp[:], uexp[:], 1.0)
            zq = const.tile([P, nchunk_b], f32)
            for it in range(6):
                src, dst = (zp, zq) if it % 2 == 0 else (zq, zp)
                nc.scalar.activation(
                    out=dst[:], in_=src[:], func=mybir.ActivationFunctionType.Sqrt,
                )
            wq = const.tile([P, nchunk_b], f32)
            nc.vector.tensor_scalar_add(wq[:], zp[:], -1.0)  # after 6 sqrts result is in zp
            ws = const.tile([P, nchunk_b], f32)
            nc.vector.tensor_scalar(
                out=ws[:], in0=wq[:], scalar1=-32.0, scalar2=64.0,
                op0=mybir.AluOpType.mult, op1=mybir.AluOpType.add,
            )
            sp = const.tile([P, nchunk_b], f32)
            nc.vector.tensor_tensor(out=sp[:], in0=wq[:], in1=ws[:], op=mybir.AluOpType.mult)
            relu_l = const.tile([P, nchunk_b], f32)
            nc.scalar.activation(out=relu_l[:], in_=l_sb[:], func=mybir.ActivationFunctionType.Relu)
            lt = const.tile([P, nchunk_b], f32)
            nc.vector.tensor_tensor(out=lt[:], in0=l_sb[:], in1=t_sb[:], op=mybir.AluOpType.mult)
            le1 = const.tile([P, nchunk_b], f32)
            nc.vector.tensor_tensor(out=le1[:], in0=relu_l[:], in1=lt[:], op=mybir.AluOpType.subtract)
            le2 = const.tile([P, nchunk_b], f32)
            nc.vector.tensor_tensor(out=le2[:], in0=le1[:], in1=sp[:], op=mybir.AluOpType.add)
            lrow = const.tile([P, 1], f32)
            nc.vector.tensor_reduce(
                out=lrow[:], in_=le2[:], axis=mybir.AxisListType.X, op=mybir.AluOpType.add,
            )
            loss_ps = psum1.tile([1, 1], f32, space="PSUM")
            nc.tensor.matmul(out=loss_ps[:], lhsT=ones_col[:], rhs=lrow[:], start=True, stop=True)
            loss_sb = const.tile([1, 1], f32)
            nc.vector.tensor_copy(loss_sb[:], loss_ps[:])
            nc.sync.dma_start(out=loss_out[:], in_=loss_sb[:])

    nc.compile()
    _PROGRAM_CACHE[key] = nc
    return nc


def _prep_core(tokens_c, lengths_c):
    """Build the index stream + per-block seg/weight maps for one core.

    Returns (regions, blocks) where regions[q] is an int16 array of local
    chunk indices (multiple of BS rows) and blocks[q] is a list of
    (seg_id, weight) per BS-row block.
    """
    n = lengths_c.shape[0]
    regions = [[] for _ in range(NCHUNK)]
    blocks = [[] for _ in range(NCHUNK)]
    for s in range(n):
        toks = tokens_c[s, :lengths_c[s]]
        w = 1.0 / float(lengths_c[s])
        q = toks // CHUNK
        loc = toks - q * CHUNK
        for qq in range(NCHUNK):
            sub = loc[q == qq]
            if sub.size == 0:
                continue
            pad = (-sub.size) % BS
            if pad:
                sub = np.concatenate([sub, np.full(pad, ZROW, dtype=sub.dtype)])
            regions[qq].append(sub.astype(np.int16))
            blocks[qq].extend([(s, w)] * (sub.size // BS))
    out_regions = []
    for qq in range(NCHUNK):
        if regions[qq]:
            out_regions.append(np.concatenate(regions[qq]))
        else:
            out_regions.append(np.zeros(0, dtype=np.int16))
    return out_regions, blocks


def prepare_all(inputs):
    """Host preprocessing: returns (compiled program, per-core input maps)."""
    tokens = np.asarray(inputs["tokens"], dtype=np.int32)
    lengths = np.asarray(inputs["lengths"], dtype=np.int32)
    t = np.asarray(inputs["t"], dtype=np.float32)
    emb_table = np.asarray(inputs["emb_table"], dtype=np.float32)
    W1 = np.asarray(inputs["W1"], dtype=np.float32)
    b1 = np.asarray(inputs["b1"], dtype=np.float32)
    gamma = np.asarray(inputs["gamma"], dtype=np.float32)
    beta = np.asarray(inputs["beta"], dtype=np.float32)
    w2 = np.asarray(inputs["w2"], dtype=np.float32)
    b2 = np.asarray(inputs["b2"], dtype=np.float32)

    # ---- per-core streams -------------------------------------------------
    per_core = []
    maxR = 0
    for c in range(NCORES):
        regions, blocks = _prep_core(
            tokens[c * BPC:(c + 1) * BPC], lengths[c * BPC:(c + 1) * BPC]
        )
        per_core.append((regions, blocks))
        maxR = max(maxR, max(r.size for r in regions))
    R = -(-maxR // ROWS_PER_ROUND) * ROWS_PER_ROUND  # round up to 1024
    NR = NCHUNK * R // ROWS_PER_ROUND

    # ---- shared table (bf16, chunked with zero rows) ----------------------
    table_bf = np.zeros((NCHUNK * CHUNK_P, D), dtype=ml_dtypes.bfloat16)
    for q in range(NCHUNK):
        table_bf[q * CHUNK_P:q * CHUNK_P + CHUNK] = emb_table[q * CHUNK:(q + 1) * CHUNK]
        # row q*CHUNK_P + CHUNK stays zero

    w1t_arr = np.ascontiguousarray(W1.T)                       # [d, d']
    w2_arr = np.ascontiguousarray(w2.reshape(2, P).T)          # [P, 2]
    gam_arr = np.ascontiguousarray(gamma.reshape(2, P).T)
    bet_arr = np.ascontiguousarray(beta.reshape(2, P).T)
    b2_arr = np.full((P, 1), float(b2), dtype=np.float32)
    iota_arr = np.tile(np.arange(BPC, dtype=np.float32), (P, 1))
    # l1 mask j: [p, m] = 1 iff m == 16*j + p//8
    l1_arr = np.zeros((P, TILES_PER_ROUND * P), dtype=ml_dtypes.bfloat16)
    pp = np.arange(P)
    for j in range(TILES_PER_ROUND):
        l1_arr[pp, j * P + BLOCKS_PER_TILE * j + pp // BS] = 1.0
    # b1 is intentionally unused: train-mode BN makes z shift-invariant.
    _ = b1

    in_maps = []
    for c in range(NCORES):
        regions, blocks = per_core[c]
        stream = np.full(NCHUNK * R, 0, dtype=np.int16)
        seg_flat = np.full(NCHUNK * R // BS, -1.0, dtype=np.float32)
        w_flat = np.zeros(NCHUNK * R // BS, dtype=np.float32)
        for q in range(NCHUNK):
            r = regions[q]
            stream[q * R:q * R + r.size] = r
            stream[q * R + r.size:(q + 1) * R] = ZROW
            nb = len(blocks[q])
            if nb:
                bs_arr = np.array(blocks[q], dtype=np.float32)
                seg_flat[q * R // BS:q * R // BS + nb] = bs_arr[:, 0]
                w_flat[q * R // BS:q * R // BS + nb] = bs_arr[:, 1]

        # idx layout: stream[i] -> partition i%16, col i//16; replicated x8
        idx_arr = np.zeros((P, NCHUNK * R // 16), dtype=np.int16)
        base16 = stream.reshape(-1, 16).T
        for rep in range(8):
            idx_arr[rep * 16:(rep + 1) * 16, :] = base16

        # block g -> psum partition q=16*(c%8)+k, round u=c//8 (c=g//16, k=g%16)
        g = np.arange(NCHUNK * R // BS)
        tile_c = g // BLOCKS_PER_TILE
        kk = g % BLOCKS_PER_TILE
        qq = BLOCKS_PER_TILE * (tile_c % TILES_PER_ROUND) + kk
        uu = tile_c // TILES_PER_ROUND
        seg_arr = np.zeros((P, NR), dtype=np.float32)
        w_arr = np.zeros((P, NR), dtype=np.float32)
        seg_arr[qq, uu] = seg_flat
        w_arr[qq, uu] = w_flat

        t_arr = np.ascontiguousarray(
            t[c * BPC:(c + 1) * BPC].reshape(BPC // P, P).T
        ).astype(np.float32)

        in_maps.append({
            "table": table_bf,
            "idx": idx_arr,
            "seg": seg_arr,
            "wts": w_arr,
            "w1t": w1t_arr,
            "w2v": w2_arr,
            "gam": gam_arr,
            "bet": bet_arr,
            "tv": t_arr,
            "b2v": b2_arr,
            "iotab": iota_arr,
            "l1m": l1_arr,
        })

    nc = _build_program(R)
    return nc, in_maps


def kernel(**inputs):
    nc, in_maps = prepare_all(inputs)
    res = run_bass_kernel_spmd(nc, in_maps, core_ids=list(range(NCORES)))
    global LAST_EXEC_NS, LAST_RESULTS
    LAST_EXEC_NS = res.exec_time_ns
    LAST_RESULTS = res

    logits = np.zeros(B, dtype=np.float32)
    loss_sum = 0.0
    for c in range(NCORES):
        r = res.results[c]
        logits[c * BPC:(c + 1) * BPC] = r["logits_s"].T.ravel()
        loss_sum += float(r["loss_s"][0, 0])
    loss = np.float32(loss_sum / B)
    return loss, logits
